# revision 1
# baseline (speedup 1.0000x reference)
"""Trainium2 Bass kernel for NeuralGraphOutput (gnn_message_passing).

Math (per sample b):
    out[b, :] = sum_a mask[b,a] * relu(cat(atoms[b,a,:], sum_d bonds[b,a,d,:]) @ W + bias)
    mask[b,a] = any(edges[b,a,:] != -1)

Strategy: pure data parallel over 8 NeuronCores (256 samples each).
Tolerance is rel_err < 2e-2, so the host casts atoms/bonds/W to bf16 and
builds one fused row tensor per core:

    fused[row, 0:64]   = atoms        (bf16)
    fused[row, 64:80]  = 0            (bond-sum landing slots)
    fused[row, 80]     = 1            (bias column)
    fused[row, 81]     = 0            (pad -> KC=82 even)
    fused[row, 82:210] = bonds d-major (bf16)

rows are permuted host-side to [chunk, partition, sub, 210] so each
partition's per-chunk DMA source is one contiguous 3360B run (>=512B
avoids the DMA read-modify-write penalty) and each 1024-row chunk is a
single DMA instruction.

Per chunk (CH=8 sub-tiles of 128 rows), software-pipelined so every
cross-engine dependency is at least one chunk old (the in-order engine
queues otherwise serialize the whole machine):
  front(T):
  - one DMA: fused rows -> fused_sb [128, 8*210]
  - bond sum over D: Pool (GPSIMD) folds 8->4->2, DVE folds 2->1 into
    fused cols 64:80 (bf16 2x_1P packing on DVE; arithmetic
    tensor_tensor is the only legal elementwise opcode on Pool)
  - PE transposes fused_sb[:, c, 0:82] (bf16, 1 cyc/row) -> psum_ct
  - DVE drains psum_ct -> catT (bf16, 2x_1P)
  mid(T - mid_delay):
  - main matmuls (bf16): psum_fp[128, 4*256] = catT.T @ W_aug
  - relu psum_fp -> relu (fp8 e4m3), whole-op alternation between Act
    and DVE (dve_frac/32 of ops on DVE) for engine balance
  back(T - mid_delay - 1):
  - fp8 DoubleRow reduction matmuls at 0.5 cyc/row: each contracts one
    sample (two sub-tiles) at once: psum_out[16, 256] +=
    sum_i mask16-pair[:, i, :].T @ relu-pair[:, i, :]
    (mask16 = one-hot-by-sample-slot masks, from edges in the prepass)
  - after 16 samples (4 chunks), Act copies psum_out -> stage, SWDGE
    drains stage -> DRAM.

The fp8 relu/reduction costs ~1e-2 max rel err (vs the 2e-2 gate);
inputs in bf16 cost ~1.5e-3.

Instruction sync-wait budget is 1 inline semaphore wait; legalize_waits
splits any surplus into standalone EventSemaphore instructions.
"""

import os
from contextlib import ExitStack

import numpy as np

import concourse.bass as bass
import concourse.mybir as mybir
import concourse.tile as tile
from concourse import masks
from concourse.bass_utils import run_bass_kernel_spmd

# Problem shapes (hardcoded per contract)
B, A, D, FA, FB, FP = 2048, 256, 8, 64, 16, 256
NCORES = 8
P = 128
CH = 8                     # sub-tiles per chunk (1024 rows)
G = 4                      # sub-tiles per psum_fp half-chunk
KC = 82                    # contract rows: 64 atoms + 16 bond sums + bias + pad
FW = KC + D * FB           # fused row width: 82 + 128 = 210
SPG = 16                   # samples accumulated per psum_out group

f32 = mybir.dt.float32
bf16 = mybir.dt.bfloat16
f8 = mybir.dt.float8e4          # e4m3: relu outputs / masks for DoubleRow
i32 = mybir.dt.int32

np_bf16 = mybir.dt.np(bf16)

# Set by kernel() after a run; test.py reads exec_time_ns / trace info.
LAST_RESULTS = None


def legalize_waits(nc, max_inline=1):
    """This toolchain's walrus accepts at most one semaphore wait inline per
    instruction (64B Events struct). Tile emits multi-wait sync_info; split
    the surplus into standalone EventSemaphore instructions just before the
    instruction on the same engine queue — identical semantics."""
    f = nc.m.functions[0]
    for bb in f.blocks:
        new = []
        for inst in bb.instructions:
            si = inst.sync_info
            waits = list(si.on_wait) if (si and si.on_wait) else []
            if len(waits) > max_inline:
                keep = waits[-max_inline:]
                moved = waits[:-max_inline]
                for k, w in enumerate(moved):
                    new.append(
                        mybir.InstEventSemaphore(
                            name=f"{inst.name}-prewait{k}",
                            ins=[],
                            outs=[],
                            sync_info=mybir.SyncInfo(on_wait=[w], on_update=[]),
                            engine=inst.engine,
                        )
                    )
                si.on_wait = keep
            new.append(inst)
        bb.instructions[:] = new


def build_nc(
    n_samples_per_core: int,
    legalize: bool = True,
    use_doublerow: bool = True,
    dve_frac: int = 12,       # relu ops on DVE per 32 (rest on Act)
    mid_delay: int = 1,       # chunks between front (transpose) and mains
    stage_on: str = "act",    # engine for the psum_out->SBUF stage copy
    fused_bufs: int = 4,
    catT_bufs: int = 3,
    relup_bufs: int = 4,
    pool_folds: int = 2,      # bond folds done on Pool (1=8->4, 2=+4->2)
    relu_halves: int = 1,     # relu ops per psum_fp tile (2 = per-reduction)
    psct_bufs: int = 2,
    psout_bufs: int = 2,
    bs_bufs: int = 2,
    drain_act: int = 0,       # catT drains on Act per 8 chunks (rest DVE)
    relu_phase: int = 1,      # phase offset of the DVE/Act relu pattern
    stages: int = 5,          # debug: 1=DMA 2=+bonds+transpose+drain 3=+mains
                              #        4=+relu 5=+reduction+out
) -> bass.Bass:
    """Build the single-core Bass program (same program runs SPMD on all cores)."""
    BC = n_samples_per_core
    N = BC * A                      # flat rows per core
    NT = N // P                     # sub-tiles
    NCH = N // (CH * P)             # chunks
    CH_PER_GROUP = SPG * A // (CH * P)   # chunks per psum_out group (4)
    assert NCH % CH_PER_GROUP == 0

    nc = bass.Bass()
    fused_d = nc.dram_tensor("fused", [NCH * P, CH * FW], bf16, kind="ExternalInput")
    edges_d = nc.dram_tensor("edges", [N, D], bf16, kind="ExternalInput")
    # host passes W stacked: rows 0:80 = W, row 80 = b, row 81 = 0 (pad)
    w_d = nc.dram_tensor("w", [KC, FP], bf16, kind="ExternalInput")
    out_d = nc.dram_tensor("out", [BC, FP], f32, kind="ExternalOutput")

    with ExitStack() as ctx:
        tc = ctx.enter_context(tile.TileContext(nc))
        singles = ctx.enter_context(tc.tile_pool(name="singles", bufs=1))

        # ---- constants ----
        w_sb = singles.tile([KC, FP], bf16)
        nc.sync.dma_start(out=w_sb[:], in_=w_d[:, :])
        # identity built on gpsimd, then laundered through DVE so consumers
        # depend on a single engine lane
        identity_src = singles.tile([P, P], bf16)
        masks.make_identity(nc, identity_src[:])
        identity = singles.tile([P, P], bf16)
        nc.vector.tensor_copy(identity[:], identity_src[:])

        # mask16[:, t, s] = mask of row t*128+p if sub-tile t belongs to
        # sample slot s of its 16-sample group, else 0. f8 so consecutive
        # sub-tile pairs feed one DoubleRow reduction matmul.
        # (zeroed via Act's u32-bitcast memzero: cheap and off DVE/Pool)
        mask16 = singles.tile([P, NT, SPG], f8)
        nc.scalar.memzero(mask16[:])

        # PSUM pool for transposes — shared (same tag) between the prepass
        # and the main loop so slot reuse is PE-internal
        psct = ctx.enter_context(tc.tile_pool(name="psct", bufs=psct_bufs, space="PSUM"))

        # ---- mask pre-pass (pool stays alive: avoids release-zone deps) ----
        # Pipelined in nblk column-blocks so the first reduction matmuls
        # aren't gated on one long serial chain.
        RPP = N // P  # rows per partition
        pp = ctx.enter_context(tc.tile_pool(name="prepass", bufs=1))
        nblk = max(RPP // P, 1)  # 4 column-blocks (1 only in scale tests)
        BR = RPP // nblk         # rows-per-partition per block
        edges_r = edges_d[:, :].rearrange("(p j r) d -> j p (r d)", p=P, j=nblk)
        # mask16 scatter: sub-tile t = 32u + 2s + h lives at flat col
        # 512u + 33s + 16h; for transpose block j (u in [uj, uj+U)) this is
        # an affine AP [p][u:U, 512][s:16, 33][h:2, 16].
        m16flat = mask16.rearrange("p t s -> p (t s)")
        maskT = pp.tile([P, NT], bf16)
        # mask = any(edge != -1) = (sum_d edges > -8): every edge >= -1, so
        # the sum is -8 iff all are -1, and bf16 rounding of a sum containing
        # a non-negative term can never land on -8. Whole chain on Pool
        # (tensor_tensor is its only legal elementwise opcode; DVE is the
        # busy engine).
        for j in range(nblk):
            edges_sb = pp.tile([P, BR * D], bf16, name=f"edges{j}")
            nc.sync.dma_start(out=edges_sb[:], in_=edges_r[j])
            eview = edges_sb.rearrange("p (r e x) -> p r e x", e=2, x=D // 2)
            e4 = pp.tile([P, BR, D // 2], bf16, name=f"e4_{j}")
            nc.gpsimd.tensor_tensor(
                out=e4[:], in0=eview[:, :, 0], in1=eview[:, :, 1],
                op=mybir.AluOpType.add,
            )
            e4v = e4.rearrange("p r (e x) -> p r e x", e=2)
            e2 = pp.tile([P, BR, D // 4], bf16, name=f"e2_{j}")
            nc.gpsimd.tensor_tensor(
                out=e2[:], in0=e4v[:, :, 0], in1=e4v[:, :, 1],
                op=mybir.AluOpType.add,
            )
            e2v = e2.rearrange("p r (e x) -> p r e x", e=2)
            e1 = pp.tile([P, BR], bf16, name=f"e1_{j}")
            nc.gpsimd.tensor_tensor(
                out=e1.rearrange("p (r x) -> p r x", x=1),
                in0=e2v[:, :, 0], in1=e2v[:, :, 1],
                op=mybir.AluOpType.add,
            )
            # compare on DVE: Pool's TensorTensor only supports arithmetic
            # ops in hardware, not comparisons
            masknat = pp.tile([P, BR], bf16, name=f"masknat{j}")
            nc.vector.tensor_scalar(
                out=masknat[:], in0=e1[:], scalar1=-float(D), scalar2=None,
                op0=mybir.AluOpType.is_gt,
            )
            if RPP < P:
                continue  # scale-test build: timing only, masks left zero
            # block j covers sub-tiles t in [j*BR, (j+1)*BR); drains + scatter
            # on Act (DVE is the saturated engine, Act has slack)
            pst = psct.tile([P, P], bf16, name="pst", tag="psum_ct")
            nc.tensor.transpose(pst[:], masknat[:], identity[:])
            nc.scalar.copy(maskT[:, j * BR : (j + 1) * BR], pst[:])
            # scatter into one-hot-by-sample-slot layout, one copy per sample
            # slot s: dst cols 512u + 33s + 16h as a composed strided view
            # [p, u, h]. Block 0 gates the first 16 chunks' reductions, so it
            # scatters immediately; blocks 1..3 are batched at the end (their
            # consumers are >=32us away).
            U = BR // (2 * SPG)
            mtw = maskT.rearrange("p (u s h) -> p u s h", s=SPG, h=2)
            m16u = mask16.rearrange("p (u w) s -> p u (w s)", w=2 * SPG)
            if RPP >= P and j == 0:
                for s in range(SPG):
                    dst = m16u[:, 0:U, 33 * s : 33 * s + 17 : SPG]
                    nc.scalar.copy(dst, mtw[:, 0:U, s, :])
        if RPP >= P and nblk > 1:
            for s in range(SPG):
                dst = m16u[:, U:, 33 * s : 33 * s + 17 : SPG]
                nc.scalar.copy(dst, mtw[:, U:, s, :])

        # ---- main loop ----
        fusedp = ctx.enter_context(tc.tile_pool(name="fusedp", bufs=fused_bufs))
        bs4p = ctx.enter_context(tc.tile_pool(name="bs4p", bufs=bs_bufs))
        bs2p = ctx.enter_context(tc.tile_pool(name="bs2p", bufs=bs_bufs))
        catTp = ctx.enter_context(tc.tile_pool(name="catTp", bufs=catT_bufs))
        relup = ctx.enter_context(tc.tile_pool(name="relup", bufs=relup_bufs))
        psfp = ctx.enter_context(tc.tile_pool(name="psfp", bufs=2, space="PSUM"))
        psout = ctx.enter_context(tc.tile_pool(name="psout", bufs=psout_bufs, space="PSUM"))
        stagep = ctx.enter_context(tc.tile_pool(name="stagep", bufs=2))

        fused_r = fused_d[:, :].rearrange("(T p) f -> T p f", p=P)

        psum_out = None

        def drain(t_id, denom):
            if t_id % denom != denom - 1:
                return
            grp = t_id // denom
            stage = stagep.tile([SPG, FP], f32)
            if stage_on == "dve":
                nc.vector.tensor_scalar(
                    out=stage[:], in0=psum_out[:],
                    scalar1=0.0, scalar2=None, op0=mybir.AluOpType.add,
                )
            else:
                nc.scalar.copy(stage[:], psum_out[:])
            # SWDGE so the output path doesn't perturb the HWDGE lane
            # rotation the input stream relies on
            nc.gpsimd.dma_start(
                out=out_d[grp * SPG : (grp + 1) * SPG, :], in_=stage[:]
            )

        # Three-stage software pipeline (engines are in-order queues, so any
        # same-chunk cross-engine chain serializes the whole machine):
        #   front(T):  DMA, bond folds, transposes, catT drain
        #   mid(T-1):  mains (reads catT drained last iteration) + relu
        #   back(T-2): reduction matmuls + psum_out drain
        # Every cross-engine dependency is >= 1 chunk old, so no engine ever
        # stalls on another engine's same-iteration work.
        front_q = []
        mid_q = []
        for T in range(NCH + mid_delay + 1):
            if T < NCH and stages >= 1:
                fused_sb = fusedp.tile([P, CH * FW], bf16)
                nc.sync.dma_start(out=fused_sb[:], in_=fused_r[T])
                fv = fused_sb.rearrange("p (c f) -> p c f", f=FW)

                if stages >= 2:
                    # bond sum over D: Pool folds 8->4 (GPSIMD is ~2.2ns/elem
                    # for real, keep its share small), DVE folds 4->2->1 with
                    # bf16 2x_1P packing
                    bview = fv[:, :, KC:FW].rearrange("p c (e x) -> p c e x", e=2)
                    bs4 = bs4p.tile([P, CH, (D // 2) * FB], bf16)
                    nc.gpsimd.tensor_tensor(
                        out=bs4[:], in0=bview[:, :, 0], in1=bview[:, :, 1],
                        op=mybir.AluOpType.add,
                    )
                    b4view = bs4.rearrange("p c (e x) -> p c e x", e=2)
                    bs2 = bs2p.tile([P, CH, (D // 4) * FB], bf16)
                    eng2 = nc.gpsimd if pool_folds >= 2 else nc.vector
                    eng2.tensor_tensor(
                        out=bs2[:], in0=b4view[:, :, 0], in1=b4view[:, :, 1],
                        op=mybir.AluOpType.add,
                    )
                    b2view = bs2.rearrange("p c (e x) -> p c e x", e=2)
                    nc.vector.tensor_tensor(
                        out=fv[:, :, FA : FA + FB],
                        in0=b2view[:, :, 0], in1=b2view[:, :, 1],
                        op=mybir.AluOpType.add,
                    )

                    # feature-major transpose via PE (bf16, 1 cyc/row)
                    psum_ct = psct.tile([KC, CH * P], bf16, tag="psum_ct")
                    for c in range(CH):
                        nc.tensor.transpose(
                            psum_ct[:, c * P : (c + 1) * P],
                            fv[:, c, 0:KC],
                            identity[:],
                        )
                    # catT drain: mostly DVE (2x_1P, 658ns) with a fraction on
                    # Act (996ns) to level the two engines' load
                    catT = catTp.tile([KC, CH * P], bf16)
                    if ((T + 1) * drain_act) // 8 != (T * drain_act) // 8:
                        nc.scalar.copy(catT[:], psum_ct[:, :])
                    else:
                        nc.vector.tensor_scalar(
                            out=catT[:], in0=psum_ct[:, :],
                            scalar1=0.0, scalar2=None, op0=mybir.AluOpType.add,
                        )

                if stages >= 3:
                    front_q.append((T, catT))

            if stages >= 3 and front_q and (T >= mid_delay or T >= NCH):
                Tm, catT_m = front_q.pop(0)
                relus = []
                for h2 in range(CH // G):
                    # main matmuls (bias folded in via ones col 80)
                    psum_fp = psfp.tile([P, G * FP], f32)
                    for g in range(G):
                        c = h2 * G + g
                        nc.tensor.matmul(
                            psum_fp[:, g * FP : (g + 1) * FP],
                            lhsT=catT_m[:, c * P : (c + 1) * P],
                            rhs=w_sb[:, :],
                            start=True,
                            stop=True,
                        )
                    if stages < 4:
                        continue
                    # relu: whole-op engine assignment (a column split makes
                    # every consumer wait on two engines and schedules worse);
                    # DVE takes dve_frac/16 of the ops, Act the rest
                    # relu_halves=2 emits two [P, 512] ops, each feeding
                    # exactly one DoubleRow reduction (finer overlap)
                    relu = relup.tile([P, G * FP], f8)
                    W2 = (G * FP) // relu_halves
                    for rh in range(relu_halves):
                        i_op = (2 * Tm + h2) * relu_halves + rh + relu_phase
                        on_dve = ((i_op + 1) * dve_frac) // 32 != (
                            (i_op * dve_frac) // 32
                        )
                        sl = slice(rh * W2, (rh + 1) * W2)
                        if on_dve:
                            nc.vector.tensor_scalar(
                                out=relu[:, sl], in0=psum_fp[:, sl],
                                scalar1=0.0, scalar2=None,
                                op0=mybir.AluOpType.max,
                            )
                        else:
                            nc.scalar.activation(
                                relu[:, sl], psum_fp[:, sl],
                                mybir.ActivationFunctionType.Relu,
                            )
                    relus.append(relu)
                if stages >= 4:
                    mid_q.append((Tm, relus))

            if stages >= 5 and mid_q and (T >= mid_delay + 1 or T >= NCH):
                Tp, relus = mid_q.pop(0)
                for h2 in range(CH // G):
                    relu = relus[h2]
                    if use_doublerow:
                        # mask-weighted atom reduction: one fp8 DoubleRow
                        # matmul per sub-tile pair (= one sample, 256 rows)
                        relu_pairs = relu.rearrange(
                            "p (g2 two f) -> p g2 two f", two=2, f=FP
                        )
                        for g2 in range(G // 2):
                            t2 = (CH * Tp + h2 * G) // 2 + g2
                            if t2 % SPG == 0:
                                psum_out = psout.tile(
                                    [SPG, FP], f32, name="psum_out"
                                )
                            nc.tensor.matmul(
                                psum_out[:, :],
                                lhsT=mask16[:, 2 * t2 : 2 * t2 + 2, :],
                                rhs=relu_pairs[:, g2],
                                start=(t2 % SPG == 0),
                                stop=(t2 % SPG == SPG - 1),
                                perf_mode=mybir.MatmulPerfMode.DoubleRow,
                            )
                            drain(t2, SPG)
                    else:
                        for g in range(G):
                            t = CH * Tp + h2 * G + g
                            if t % (2 * SPG) == 0:
                                psum_out = psout.tile(
                                    [SPG, FP], f32, name="psum_out"
                                )
                            nc.tensor.matmul(
                                psum_out[:, :],
                                lhsT=mask16[:, t, :],
                                rhs=relu[:, g * FP : (g + 1) * FP],
                                start=(t % (2 * SPG) == 0),
                                stop=(t % (2 * SPG) == 2 * SPG - 1),
                            )
                            drain(t, 2 * SPG)
    if legalize:
        legalize_waits(nc)
    return nc


def stack_w(W, b):
    """Host-side W layout matching catT rows: W | bias | zero pad (bf16)."""
    return np.ascontiguousarray(
        np.vstack(
            [
                np.asarray(W, dtype=np.float32),
                np.asarray(b, dtype=np.float32).reshape(1, FP),
                np.zeros((1, FP), dtype=np.float32),
            ]
        ).astype(np_bf16)
    )


def make_fused(atoms_flat, bonds_flat):
    """Build the fused [N, 210] bf16 tensor and permute to the DMA layout
    [NCH*P, CH*FW] where each partition's chunk data is contiguous.

    atoms_flat: [N, 64] f32/bf16, bonds_flat: [N, 128] f32/bf16.
    Row order: flat row = T*CH*P + c*P + p  ->  dram[(T, p), (c, f)].
    """
    N = atoms_flat.shape[0]
    NCH = N // (CH * P)
    fused = np.zeros((N, FW), dtype=np_bf16)
    fused[:, 0:FA] = atoms_flat.astype(np_bf16)
    fused[:, FA + FB] = 1.0
    fused[:, KC:FW] = bonds_flat.astype(np_bf16)
    perm = fused.reshape(NCH, CH, P, FW).transpose(0, 2, 1, 3)
    return np.ascontiguousarray(perm.reshape(NCH * P, CH * FW))


def _shard_inputs(atoms, bonds, edges, W, b, n_samples_per_core):
    BC = n_samples_per_core
    N = BC * A
    in_maps = []
    w_np = stack_w(W, b)
    atoms = np.asarray(atoms, dtype=np.float32)
    bonds = np.asarray(bonds, dtype=np.float32)
    edges = np.asarray(edges, dtype=np.int32)
    for c in range(NCORES):
        sl = slice(c * BC, (c + 1) * BC)
        in_maps.append(
            {
                "fused": make_fused(
                    atoms[sl].reshape(N, FA), bonds[sl].reshape(N, D * FB)
                ),
                "edges": np.ascontiguousarray(edges[sl].reshape(N, D).astype(np_bf16)),
                "w": w_np,
            }
        )
    return in_maps


def kernel(atoms, bonds, edges, W, b):
    """Full inputs in, full output out. Shards batch across 8 cores."""
    global LAST_RESULTS
    BC = B // NCORES
    nc = build_nc(BC)
    in_maps = _shard_inputs(atoms, bonds, edges, W, b, BC)
    core_ids = list(range(NCORES))
    trace = bool(os.environ.get("KERNEL_TRACE"))
    res = run_bass_kernel_spmd(nc, in_maps, core_ids, trace=trace)
    LAST_RESULTS = res
    out = np.concatenate([res.results[c]["out"] for c in range(NCORES)], axis=0)
    return out.astype(np.float32)



# revision 9
# speedup vs baseline: 1.6417x; 1.6417x over previous
"""Trainium2 Bass kernel for NeuralGraphOutput (gnn_message_passing).

Math (per sample b):
    out[b, :] = sum_a mask[b,a] * relu(cat(atoms[b,a,:], sum_d bonds[b,a,d,:]) @ W + bias)
    mask[b,a] = any(edges[b,a,:] != -1)

Strategy: pure data parallel over 8 NeuronCores (256 samples each).

Key structure: the bond sum over D is folded INTO the PE contraction by
replicating the bond weight rows 8x, so the per-atom dense becomes ONE
fp8 DoubleRow matmul with K=256 logical rows (128 partitions x 2):

    rows   0:64   atoms                          W rows: fp8(W_a)
    rows  64:192  bonds, d-major (64+16d+f)      W rows: fp8(W_b) tiled 8x
    row  192      ones (bias column)             W row:  b
    rows 193:209  bond-sum/32 correction channel W rows: 32*(W_b - fp8(W_b))
    rows 209:256  atoms[rank]/32 correction      W rows: 32*(W_a - fp8(W_a))[rank]

The correction channels cancel the fp8 weight-quantization error (which
is otherwise correlated across all atoms and, for bonds, multiplied by
the bond sum): total rel err ~1.2e-2 vs the 2e-2 gate. All feature rows
are packed host-side in feature-major DoubleRow layout (pair dim r=2p+j),
so the device does NO transposes, NO bond folds and NO catT drains --
per 1024-row chunk the device work is just:
    1 DMA  [128, 8*256] fp8 (256 KB, full partition fan-out)
    8 DoubleRow mains   psum_fp[128, 256] = data[128,2,128].T @ wc[128,2,256]
    2 relu ops (Act/DVE split)  psum -> fp8
    4 DoubleRow reductions psum_out[16,256] += mask16-pair.T @ relu-pair
mask16 = one-hot-by-sample-slot masks built from edges in a prepass
(unchanged from the previous kernel generation).

Instruction sync-wait budget is 1 inline semaphore wait; legalize_waits
splits any surplus into standalone EventSemaphore instructions.
"""

import os
from contextlib import ExitStack

import numpy as np

import concourse.bass as bass
import concourse.mybir as mybir
import concourse.tile as tile
from concourse import masks
from concourse.bass_utils import run_bass_kernel_spmd

# Problem shapes (hardcoded per contract)
B, A, D, FA, FB, FP = 2048, 256, 8, 64, 16, 256
NCORES = 8
P = 128
CH = 8                     # sub-tiles per chunk (1024 rows)
G = 4                      # sub-tiles per psum_fp half-chunk
KP = 128                   # contract partitions (DoubleRow: 2 rows each)
KL = 2 * KP                # logical contract rows = 256
SPG = 16                   # samples accumulated per psum_out group
SC = 32.0                  # correction-channel scale
NB_CORR = 16               # bond-sum correction rows
NA_CORR = KL - (FA + D * FB + 1) - NB_CORR  # 47 atom correction rows

f32 = mybir.dt.float32
bf16 = mybir.dt.bfloat16
f8 = mybir.dt.float8e4          # e4m3
i32 = mybir.dt.int32

np_bf16 = mybir.dt.np(bf16)
np_f8 = mybir.dt.np(f8)

# Set by kernel() after a run; test.py reads exec_time_ns / trace info.
LAST_RESULTS = None


def legalize_waits(nc, max_inline=1):
    """This toolchain's walrus accepts at most one semaphore wait inline per
    instruction (64B Events struct). Tile emits multi-wait sync_info; split
    the surplus into standalone EventSemaphore instructions just before the
    instruction on the same engine queue -- identical semantics."""
    f = nc.m.functions[0]
    for bb in f.blocks:
        new = []
        for inst in bb.instructions:
            si = inst.sync_info
            waits = list(si.on_wait) if (si and si.on_wait) else []
            if len(waits) > max_inline:
                keep = waits[-max_inline:]
                moved = waits[:-max_inline]
                for k, w in enumerate(moved):
                    new.append(
                        mybir.InstEventSemaphore(
                            name=f"{inst.name}-prewait{k}",
                            ins=[],
                            outs=[],
                            sync_info=mybir.SyncInfo(on_wait=[w], on_update=[]),
                            engine=inst.engine,
                        )
                    )
                si.on_wait = keep
            new.append(inst)
        bb.instructions[:] = new
    return nc


def build_nc(
    n_samples_per_core: int,
    legalize: bool = True,
    dve_frac: int = 31,       # relu ops on DVE per 64 (rest on Act)
    mid_delay: int = 1,       # chunks between DMA and mains
    stage_on: str = "act",    # engine for the psum_out->SBUF stage copy
    data_bufs: int = 4,
    relup_bufs: int = 4,
    psfp_bufs: int = 3,
    psout_bufs: int = 2,
    scatter_on: str = "pool", # engine for the mask16 scatter copies
    relu_halves: int = 1,     # relu ops per psum_fp tile
    relu_phase: int = 1,      # phase offset of the DVE/Act relu pattern
) -> bass.Bass:
    """Build the single-core Bass program (same program runs SPMD on all cores)."""
    BC = n_samples_per_core
    N = BC * A                      # flat rows per core
    NT = N // P                     # sub-tiles
    NCH = N // (CH * P)             # chunks
    CH_PER_GROUP = SPG * A // (CH * P)   # chunks per psum_out group (4)
    assert NCH % CH_PER_GROUP == 0

    nc = bass.Bass()
    data_d = nc.dram_tensor("data", [NCH * KP, CH * 2 * P], f8, kind="ExternalInput")
    edges_d = nc.dram_tensor("edges", [N, D], bf16, kind="ExternalInput")
    wc_d = nc.dram_tensor("wc", [KP, 2 * FP], f8, kind="ExternalInput")
    out_d = nc.dram_tensor("out", [BC, FP], f32, kind="ExternalOutput")

    with ExitStack() as ctx:
        tc = ctx.enter_context(tile.TileContext(nc))
        singles = ctx.enter_context(tc.tile_pool(name="singles", bufs=1))

        # ---- constants ----
        # constants + prepass DMAs ride the Act HWDGE queue so the SP
        # queue's first entries are the chunk-0/1 data DMAs
        wc_sb = singles.tile([KP, 2 * FP], f8)
        nc.scalar.dma_start(out=wc_sb[:], in_=wc_d[:, :])
        wcv = wc_sb.rearrange("k (j n) -> k j n", j=2)
        # identity built on gpsimd, then laundered through DVE so consumers
        # depend on a single engine lane
        identity_src = singles.tile([P, P], bf16)
        masks.make_identity(nc, identity_src[:])
        identity = singles.tile([P, P], bf16)
        nc.vector.tensor_copy(identity[:], identity_src[:])

        # mask16[:, t, s] = mask of row t*128+p if sub-tile t belongs to
        # sample slot s of its 16-sample group, else 0. f8 so consecutive
        # sub-tile pairs feed one DoubleRow reduction matmul.
        mask16 = singles.tile([P, NT, SPG], f8)
        nc.gpsimd.memzero(mask16[:])

        # psfp doubles as the prepass-transpose pool (its [P, G*FP] f32
        # buffers dwarf the [P, P] bf16 prepass tiles): 3 bufs x 2 banks +
        # psout 2 x 1 bank = exactly the 8 PSUM banks.
        psfp = ctx.enter_context(tc.tile_pool(name="psfp", bufs=psfp_bufs, space="PSUM"))

        # ---- mask pre-pass (pool stays alive: avoids release-zone deps) ----
        RPP = N // P  # rows per partition
        pp = ctx.enter_context(tc.tile_pool(name="prepass", bufs=1))
        nblk = max(RPP // P, 1)  # 4 column-blocks (1 only in scale tests)
        BR = RPP // nblk         # rows-per-partition per block
        edges_r = edges_d[:, :].rearrange("(p j r) d -> j p (r d)", p=P, j=nblk)
        m16flat = mask16.rearrange("p t s -> p (t s)")
        maskT = pp.tile([P, NT], bf16)
        # mask = any(edge != -1) = (sum_d edges > -8): every edge >= -1, so
        # the sum is -8 iff all are -1, and bf16 rounding of a sum containing
        # a non-negative term can never land on -8. Whole chain on Pool.
        for j in range(nblk):
            edges_sb = pp.tile([P, BR * D], bf16, name=f"edges{j}")
            nc.scalar.dma_start(out=edges_sb[:], in_=edges_r[j])
            eview = edges_sb.rearrange("p (r e x) -> p r e x", e=2, x=D // 2)
            e4 = pp.tile([P, BR, D // 2], bf16, name=f"e4_{j}")
            nc.gpsimd.tensor_tensor(
                out=e4[:], in0=eview[:, :, 0], in1=eview[:, :, 1],
                op=mybir.AluOpType.add,
            )
            e4v = e4.rearrange("p r (e x) -> p r e x", e=2)
            e2 = pp.tile([P, BR, D // 4], bf16, name=f"e2_{j}")
            nc.gpsimd.tensor_tensor(
                out=e2[:], in0=e4v[:, :, 0], in1=e4v[:, :, 1],
                op=mybir.AluOpType.add,
            )
            e2v = e2.rearrange("p r (e x) -> p r e x", e=2)
            e1 = pp.tile([P, BR], bf16, name=f"e1_{j}")
            nc.gpsimd.tensor_tensor(
                out=e1.rearrange("p (r x) -> p r x", x=1),
                in0=e2v[:, :, 0], in1=e2v[:, :, 1],
                op=mybir.AluOpType.add,
            )
            # compare on DVE: Pool's TensorTensor only supports arithmetic
            masknat = pp.tile([P, BR], bf16, name=f"masknat{j}")
            nc.vector.tensor_scalar(
                out=masknat[:], in0=e1[:], scalar1=-float(D), scalar2=None,
                op0=mybir.AluOpType.is_gt,
            )
            if RPP < P:
                continue  # scale-test build: timing only, masks left zero
            pst = psfp.tile([P, P], bf16, name="pst", tag="psum_fp")
            nc.tensor.transpose(pst[:], masknat[:], identity[:])
            nc.scalar.copy(maskT[:, j * BR : (j + 1) * BR], pst[:])
            # scatter into one-hot-by-sample-slot layout, one copy per sample
            # slot s -- on Pool (idle after the folds) so the Act queue isn't
            # clogged ahead of the main-loop relus. Block 0 gates the first
            # 16 chunks' reductions, so it scatters immediately; blocks 1..3
            # are batched at the end.
            scat = (
                (lambda d, s: nc.gpsimd.tensor_copy(d, s))
                if scatter_on == "pool"
                else (lambda d, s: nc.scalar.copy(d, s))
            )
            U = BR // (2 * SPG)
            mtw = maskT.rearrange("p (u s h) -> p u s h", s=SPG, h=2)
            m16u = mask16.rearrange("p (u w) s -> p u (w s)", w=2 * SPG)
            if RPP >= P and j == 0:
                for s in range(SPG):
                    dst = m16u[:, 0:U, 33 * s : 33 * s + 17 : SPG]
                    scat(dst, mtw[:, 0:U, s, :])
        if RPP >= P and nblk > 1:
            for s in range(SPG):
                dst = m16u[:, U:, 33 * s : 33 * s + 17 : SPG]
                scat(dst, mtw[:, U:, s, :])

        # ---- main loop ----
        datap = ctx.enter_context(tc.tile_pool(name="datap", bufs=data_bufs))
        relup = ctx.enter_context(tc.tile_pool(name="relup", bufs=relup_bufs))
        psout = ctx.enter_context(tc.tile_pool(name="psout", bufs=psout_bufs, space="PSUM"))
        stagep = ctx.enter_context(tc.tile_pool(name="stagep", bufs=2))

        data_r = data_d[:, :].rearrange("(T k) f -> T k f", k=KP)

        psum_out = None

        def drain(t_id, denom):
            if t_id % denom != denom - 1:
                return
            grp = t_id // denom
            stage = stagep.tile([SPG, FP], f32)
            if stage_on == "dve":
                nc.vector.tensor_scalar(
                    out=stage[:], in0=psum_out[:],
                    scalar1=0.0, scalar2=None, op0=mybir.AluOpType.add,
                )
            else:
                nc.scalar.copy(stage[:], psum_out[:])
            # SWDGE so the output path doesn't perturb the HWDGE lane
            # rotation the input stream relies on
            nc.gpsimd.dma_start(
                out=out_d[grp * SPG : (grp + 1) * SPG, :], in_=stage[:]
            )

        # Software pipeline: DMA(T) || mains+relu(T-mid_delay) ||
        # reductions(T-mid_delay-1). Every cross-engine dependency is >= 1
        # chunk old, so the in-order engine queues never stall on
        # same-iteration work.
        front_q = []
        mid_q = []
        for T in range(NCH + mid_delay + 1):
            if T < NCH:
                data_sb = datap.tile([KP, CH * 2 * P], f8)
                nc.sync.dma_start(out=data_sb[:], in_=data_r[T])
                front_q.append((T, data_sb))

            if front_q and (T >= mid_delay or T >= NCH):
                Tm, data_m = front_q.pop(0)
                dv = data_m.rearrange("k (c j m) -> k c j m", j=2, m=P)
                relus = []
                for h2 in range(CH // G):
                    psum_fp = psfp.tile([P, G * FP], f32)
                    for g in range(G):
                        c = h2 * G + g
                        nc.tensor.matmul(
                            psum_fp[:, g * FP : (g + 1) * FP],
                            lhsT=dv[:, c],
                            rhs=wcv[:, :, :],
                            start=True,
                            stop=True,
                            perf_mode=mybir.MatmulPerfMode.DoubleRow,
                        )
                    # relu: whole-op engine assignment; DVE takes dve_frac/32
                    # of the ops, Act the rest
                    relu = relup.tile([P, G * FP], f8)
                    W2 = (G * FP) // relu_halves
                    for rh in range(relu_halves):
                        i_op = (2 * Tm + h2) * relu_halves + rh + relu_phase
                        on_dve = ((i_op + 1) * dve_frac) // 64 != (
                            (i_op * dve_frac) // 64
                        )
                        sl = slice(rh * W2, (rh + 1) * W2)
                        if on_dve:
                            nc.vector.tensor_scalar(
                                out=relu[:, sl], in0=psum_fp[:, sl],
                                scalar1=0.0, scalar2=None,
                                op0=mybir.AluOpType.max,
                            )
                        else:
                            nc.scalar.activation(
                                relu[:, sl], psum_fp[:, sl],
                                mybir.ActivationFunctionType.Relu,
                            )
                    relus.append(relu)
                mid_q.append((Tm, relus))

            if mid_q and (T >= mid_delay + 1 or T >= NCH):
                Tp, relus = mid_q.pop(0)
                for h2 in range(CH // G):
                    relu = relus[h2]
                    # mask-weighted atom reduction: one fp8 DoubleRow matmul
                    # per sub-tile pair (= one sample, 256 rows)
                    relu_pairs = relu.rearrange(
                        "p (g2 two f) -> p g2 two f", two=2, f=FP
                    )
                    for g2 in range(G // 2):
                        t2 = (CH * Tp + h2 * G) // 2 + g2
                        if t2 % SPG == 0:
                            psum_out = psout.tile([SPG, FP], f32, name="psum_out")
                        nc.tensor.matmul(
                            psum_out[:, :],
                            lhsT=mask16[:, 2 * t2 : 2 * t2 + 2, :],
                            rhs=relu_pairs[:, g2],
                            start=(t2 % SPG == 0),
                            stop=(t2 % SPG == SPG - 1),
                            perf_mode=mybir.MatmulPerfMode.DoubleRow,
                        )
                        drain(t2, SPG)
    if legalize:
        legalize_waits(nc)
    return nc


# ---- host-side packing -------------------------------------------------

_F8_GRID = None


def _f8_grid():
    global _F8_GRID
    if _F8_GRID is None:
        bits = np.arange(256, dtype=np.uint8).view(np_f8).astype(np.float32)
        _F8_GRID = np.unique(np.sort(bits[np.isfinite(bits)]))
    return _F8_GRID


def _atom_rank():
    """Which atom feature rows get a correction channel (worst fp8 residual).
    Depends only on W, deterministic."""
    return None  # computed in make_wc; stored module-global for make_data


_RANK = None


def make_wc(W, b):
    """Build the [KP, 2*FP] fp8 weight tensor with correction channels.

    Row map (logical row r = 2p+j):
        0:64     fp8(W_a)
        64:192   fp8(W_b) tiled 8x (bond replicas, nearest)
        192      b (bias row; data carries ones)
        193:209  32*(W_b - fp8(W_b))     (bond-sum/32 correction channel)
        209:256  32*(W_a - fp8(W_a))[rank]  (atom correction channels)
    """
    global _RANK
    W = np.asarray(W, dtype=np.float32)
    b = np.asarray(b, dtype=np.float32).reshape(FP)
    Wa, Wb = W[0:FA], W[FA : FA + FB]
    Wa1 = Wa.astype(np_f8).astype(np.float32)
    Wb1 = Wb.astype(np_f8).astype(np.float32)
    da = Wa - Wa1
    db = Wb - Wb1
    _RANK = np.argsort(-(da * da).sum(1))[:NA_CORR]

    W_aug = np.zeros((KL, FP), dtype=np.float32)
    W_aug[0:FA] = Wa1
    W_aug[FA : FA + D * FB] = np.tile(Wb1, (D, 1))
    W_aug[FA + D * FB] = b
    W_aug[193 : 193 + NB_CORR] = db * SC
    W_aug[193 + NB_CORR :] = da[_RANK] * SC
    wc = W_aug.astype(np_f8).reshape(KP, 2 * FP)
    return np.ascontiguousarray(wc)


def make_data(atoms_flat, bonds_flat):
    """Build the [NCH*KP, CH*2*P] fp8 data tensor in DoubleRow feature-major
    layout: data[(T, p), (c, j, m)] = feat_row(2p+j) of flat row (T*CH+c)*P+m.

    atoms_flat: [N, 64] f32, bonds_flat: [N, 128] f32 (d-major).
    Requires make_wc() to have been called (sets _RANK).
    """
    N = atoms_flat.shape[0]
    NCH = N // (CH * P)
    ft = np.zeros((KL, N), dtype=np_f8)
    ft[0:FA] = atoms_flat.T.astype(np_f8)
    ft[FA : FA + D * FB] = bonds_flat.T.astype(np_f8)
    ft[FA + D * FB] = np_f8(1.0)
    bsum = bonds_flat.reshape(N, D, FB).sum(1)
    ft[193 : 193 + NB_CORR] = (bsum.T / SC).astype(np_f8)
    ft[193 + NB_CORR :] = (atoms_flat.T[_RANK] / SC).astype(np_f8)
    v = ft.reshape(KP, 2, NCH, CH, P)
    data = v.transpose(2, 0, 3, 1, 4).reshape(NCH * KP, CH * 2 * P)
    return np.ascontiguousarray(data)


def _shard_inputs(atoms, bonds, edges, W, b, n_samples_per_core):
    BC = n_samples_per_core
    N = BC * A
    in_maps = []
    wc = make_wc(W, b)
    atoms = np.asarray(atoms, dtype=np.float32)
    bonds = np.asarray(bonds, dtype=np.float32)
    edges = np.asarray(edges, dtype=np.int32)
    for c in range(NCORES):
        sl = slice(c * BC, (c + 1) * BC)
        in_maps.append(
            {
                "data": make_data(
                    atoms[sl].reshape(N, FA), bonds[sl].reshape(N, D * FB)
                ),
                "edges": np.ascontiguousarray(edges[sl].reshape(N, D).astype(np_bf16)),
                "wc": wc,
            }
        )
    return in_maps


def kernel(atoms, bonds, edges, W, b):
    """Full inputs in, full output out. Shards batch across 8 cores."""
    global LAST_RESULTS
    BC = B // NCORES
    nc = build_nc(BC)
    in_maps = _shard_inputs(atoms, bonds, edges, W, b, BC)
    core_ids = list(range(NCORES))
    trace = bool(os.environ.get("KERNEL_TRACE"))
    res = run_bass_kernel_spmd(nc, in_maps, core_ids, trace=trace)
    LAST_RESULTS = res
    out = np.concatenate([res.results[c]["out"] for c in range(NCORES)], axis=0)
    return out.astype(np.float32)
